# revision 8
# baseline (speedup 1.0000x reference)
"""AmplifiedAttention Trainium2 kernel (8 NeuronCores, SPMD).

Key algebraic simplification: rotate_half(q)·rotate_half(k) == q·k and
(rotate_half(q)^2)·(rotate_half(k)^2) == q^2·k^2, so the reference's second
"rotated" attention pass is bit-for-bit (up to fp assoc.) identical to the
first: out = out1 + HADAMARD_SCALE * out1^2 * gate_w.

Sharding: 16 heads -> 2 heads/core. Each core projects Q/K/V for its heads
over the full (batch*seq) axis, runs causal attention with a fused
second-order score term ([rope(Q); sqrt(lam)*rope(Q)^2] contraction), then an
AllToAll redistributes attention outputs so each core computes a 512-row
slice of the final output projection against the full Wo.
"""

import math
import os

import numpy as np
import ml_dtypes

import concourse.bass as bass
import concourse.bacc as bacc
import concourse.mybir as mybir
from concourse.tile import TileContext
from concourse.bass_utils import run_bass_kernel_spmd

BF16 = mybir.dt.bfloat16
F32 = mybir.dt.float32

B, S, D = 2, 2048, 1024
H = 16
HD = D // H            # 64
NC = 8                 # cores
NHPC = H // NC         # 2 heads per core
SEQ = B * S            # 4096
NK = D // 128          # 8 contraction chunks
NNT = SEQ // 512       # 8 seq 512-tiles
NST = SEQ // 128       # 32 seq 128-tiles
NQT = S // 512         # 4 query 512-tiles per (b,h)
NTC = S // 128         # 16 key 128-chunks per (b,h)

LAMBDA = 0.1
HADAMARD_SCALE = 0.05
ROPE_BASE = 10000.0
INV_SQRT_HD = 1.0 / math.sqrt(HD)
# Q weights are pre-scaled by INV_SQRT_HD on the host.  Q-tilde bottom rows
# need sqrt(lam)*INV_SQRT_HD*ropeQ^2 computed from the pre-scaled ropeQ:
#   (lam^0.25 / sqrt(INV_SQRT_HD) * x)^2 = sqrt(lam)/INV_SQRT_HD * x^2
Q_SQ_SCALE = LAMBDA ** 0.25 / math.sqrt(INV_SQRT_HD)
K_SQ_SCALE = LAMBDA ** 0.25

_GRAPH = None


def _emit(nc, tc, t):
    """Emit the per-core program. t: dict name -> DRAM tensor handle."""
    AF = mybir.ActivationFunctionType
    OP = mybir.AluOpType
    singles = tc.alloc_tile_pool(name="singles", bufs=1)

    # ---- constant / input loads ----
    cos_sb = singles.tile([128, SEQ], BF16, tag="cos", name="cos")
    sin_sb = singles.tile([128, SEQ], BF16, tag="sin", name="sin")
    masks_sb = singles.tile([128, 4, 512], BF16, tag="masks", name="masks")
    gate_sb = singles.tile([64, 1], F32, tag="gate", name="gate")
    wq_sb = singles.tile([128, NK, 256], BF16, tag="wq", name="wq")
    wk_sb = singles.tile([128, NK, 256], BF16, tag="wk", name="wk")
    wv_sb = singles.tile([128, NK, 128], BF16, tag="wv", name="wv")
    wo_sb = singles.tile([128, NK, 1024], BF16, tag="wo", name="wo")
    xt_sb = singles.tile([128, NK, SEQ], BF16, tag="xt", name="xt")

    nc.sync.dma_start(out=cos_sb[:], in_=t["cost"].ap())
    nc.sync.dma_start(out=sin_sb[:], in_=t["sint"].ap())
    nc.sync.dma_start(out=masks_sb[:], in_=t["masks"].ap().rearrange("p (o q) -> p o q", o=4))
    nc.sync.dma_start(out=gate_sb[:], in_=t["gate"].ap())
    nc.sync.dma_start(out=wq_sb[:], in_=t["wq2"].ap().rearrange("k p m -> p k m"))
    nc.sync.dma_start(out=wk_sb[:], in_=t["wk2"].ap().rearrange("k p m -> p k m"))
    nc.sync.dma_start(out=wv_sb[:], in_=t["wv"].ap().rearrange("k p m -> p k m"))
    nc.sync.dma_start(out=wo_sb[:], in_=t["wo"].ap().rearrange("k p j -> p k j"))
    for k in range(NK):
        nc.sync.dma_start(out=xt_sb[:, k, :], in_=t["xt"].ap()[k])

    # ---- persistent stage-1 outputs ----
    # qt/kt per (b, h): [128, S]: rows 0:64 = rope (Q pre-scaled by 1/sqrt(hd)),
    # rows 64:128 = scaled square.
    qt = [[singles.tile([128, S], BF16, tag=f"qt{b}{h}", name=f"qt{b}{h}") for h in range(NHPC)] for b in range(B)]
    kt = [[singles.tile([128, S], BF16, tag=f"kt{b}{h}", name=f"kt{b}{h}") for h in range(NHPC)] for b in range(B)]
    # v_sb: [128, NST, 256]: per 128-seq-chunk: [ones(0:64), h0(64:128), ones(128:192), h1(192:256)]
    v_sb = singles.tile([128, NST, 256], BF16, tag="v", name="v")
    att_sb = singles.tile([128, SEQ], BF16, tag="att", name="att")

    ones_ap = bass.AP(
        tensor=v_sb.tensor, offset=v_sb.offset,
        ap=[v_sb.ap[0], [256, NST], [128, 2], [1, 64]],
    )
    nc.vector.memset(ones_ap, 1.0)

    # ---- stage 1: projections + rope + squares + V ----
    ps_proj = tc.alloc_tile_pool(name="ps_proj", bufs=6, space="PSUM")
    ps_v = tc.alloc_tile_pool(name="ps_v", bufs=2, space="PSUM")
    rope_tmp = tc.alloc_tile_pool(name="rope_tmp", bufs=4)

    for n in range(NNT):
        ns = slice(512 * n, 512 * n + 512)
        b, j2 = n // NQT, n % NQT
        cs = slice(512 * j2, 512 * j2 + 512)
        for (w2, dst, ss) in ((wq_sb, qt, None), (wk_sb, kt, None)):
            ps_a = ps_proj.tile([128, 512], F32, tag="pp", name="pp")
            ps_s = ps_proj.tile([128, 512], F32, tag="pp", name="pp")
            for k in range(NK):
                nc.tensor.matmul(ps_a[:], w2[:, k, 0:128], xt_sb[:, k, ns],
                                 start=(k == 0), stop=(k == NK - 1))
            for k in range(NK):
                nc.tensor.matmul(ps_s[:], w2[:, k, 128:256], xt_sb[:, k, ns],
                                 start=(k == 0), stop=(k == NK - 1))
            pc = rope_tmp.tile([128, 512], BF16, tag="pc", name="pc")
            psn = rope_tmp.tile([128, 512], BF16, tag="psn", name="psn")
            nc.vector.tensor_tensor(out=pc[:], in0=ps_a[:], in1=cos_sb[:, ns], op=OP.mult)
            nc.vector.tensor_tensor(out=psn[:], in0=ps_s[:], in1=sin_sb[:, ns], op=OP.mult)
            nc.vector.tensor_tensor(out=dst[b][0][0:64, cs], in0=pc[0:64, :], in1=psn[0:64, :], op=OP.add)
            nc.vector.tensor_tensor(out=dst[b][1][0:64, cs], in0=pc[64:128, :], in1=psn[64:128, :], op=OP.add)
        # V for the 4 seq-128-chunks of this n-tile
        for st4 in range(4):
            st = 4 * n + st4
            pv = ps_v.tile([128, 128], F32, tag="pv", name="pv")
            for k in range(NK):
                nc.tensor.matmul(pv[:], xt_sb[:, k, 128 * st:128 * st + 128], wv_sb[:, k, :],
                                 start=(k == 0), stop=(k == NK - 1))
            vdst = bass.AP(
                tensor=v_sb.tensor, offset=v_sb.offset + 256 * st + 64,
                ap=[v_sb.ap[0], [128, 2], [1, 64]],
            )
            nc.scalar.copy(vdst, pv[:].rearrange("p (a b) -> p a b", a=2))

    for b in range(B):
        for h in range(NHPC):
            nc.scalar.activation(out=qt[b][h][64:128, :], in_=qt[b][h][0:64, :],
                                 func=AF.Square, scale=Q_SQ_SCALE)
            nc.scalar.activation(out=kt[b][h][64:128, :], in_=kt[b][h][0:64, :],
                                 func=AF.Square, scale=K_SQ_SCALE)

    rope_tmp.release()
    ps_v.release()
    ps_proj.release()

    # ---- stage 2: attention ----
    ps_s = tc.alloc_tile_pool(name="ps_s", bufs=3, space="PSUM")
    ps_av = tc.alloc_tile_pool(name="ps_av", bufs=2, space="PSUM")
    a_pool = tc.alloc_tile_pool(name="a_pool", bufs=4)
    nrm = tc.alloc_tile_pool(name="nrm", bufs=3)

    for b in range(B):
        for h in range(NHPC):
            QT, KT = qt[b][h], kt[b][h]
            for j in range(NQT):
                qs = slice(512 * j, 512 * j + 512)
                I = 4 * j + 4  # t-chunks (causal keep)
                po = ps_av.tile([128, 512], F32, tag="po", name="po")
                for ip in range(I // 2):
                    i0, i1 = 2 * ip, 2 * ip + 1
                    pss = ps_s.tile([128, 1024], F32, tag="pss", name="pss")
                    nc.tensor.matmul(pss[:, 0:512], KT[:, 128 * i0:128 * i0 + 128], QT[:, qs],
                                     start=True, stop=True)
                    nc.tensor.matmul(pss[:, 512:1024], KT[:, 128 * i1:128 * i1 + 128], QT[:, qs],
                                     start=True, stop=True)
                    a = a_pool.tile([128, 1024], BF16, tag="a", name="a")
                    nc.scalar.activation(out=a[:], in_=pss[:], func=AF.Exp)
                    if i0 >= 4 * j:  # diagonal pair -> apply causal masks
                        nc.vector.tensor_tensor(out=a[:, 0:512], in0=a[:, 0:512],
                                                in1=masks_sb[:, i0 - 4 * j, :], op=OP.mult)
                        nc.vector.tensor_tensor(out=a[:, 512:1024], in0=a[:, 512:1024],
                                                in1=masks_sb[:, i1 - 4 * j, :], op=OP.mult)
                    nc.tensor.matmul(po[:], v_sb[:, 16 * b + i0, 128 * h:128 * h + 128], a[:, 0:512],
                                     start=(ip == 0), stop=False)
                    nc.tensor.matmul(po[:], v_sb[:, 16 * b + i1, 128 * h:128 * h + 128], a[:, 512:1024],
                                     start=False, stop=(ip == I // 2 - 1))
                # rows 0:64 of po = softmax denominators (replicated), 64:128 = A@V
                rd = nrm.tile([64, 512], F32, tag="rd", name="rd")
                m = nrm.tile([64, 512], BF16, tag="m", name="m")
                sq = nrm.tile([64, 512], BF16, tag="sq", name="sq")
                nc.vector.reciprocal_approx_fast(out=rd[:], in_=po[0:64, :])
                nc.vector.tensor_tensor(out=m[:], in0=po[64:128, :], in1=rd[:], op=OP.mult)
                nc.vector.tensor_tensor(out=sq[:], in0=m[:], in1=m[:], op=OP.mult)
                nc.vector.scalar_tensor_tensor(
                    out=att_sb[64 * h:64 * h + 64, 2048 * b + 512 * j:2048 * b + 512 * j + 512],
                    in0=sq[:], scalar=gate_sb[:, 0:1], in1=m[:],
                    op0=OP.mult, op1=OP.add)

    nrm.release()
    a_pool.release()
    ps_av.release()
    ps_s.release()

    # ---- stage 3: AllToAll + output projection ----
    nc.sync.dma_start(
        out=t["cc_in"].ap().rearrange("(c p) q -> p c q", p=128),
        in_=att_sb[:].rearrange("p (c q) -> p c q", q=512),
    )
    nc.gpsimd.collective_compute(
        "AllToAll", OP.bypass,
        replica_groups=[list(range(NC))],
        ins=[t["cc_in"].ap()], outs=[t["cc_out"].ap()],
    )
    ga_sb = singles.tile([128, NK, 512], BF16, tag="ga", name="ga")
    nc.sync.dma_start(out=ga_sb[:], in_=t["cc_out"].ap().rearrange("(k p) q -> p k q", p=128))

    ps_o = tc.alloc_tile_pool(name="ps_o", bufs=2, space="PSUM")
    ob = tc.alloc_tile_pool(name="ob", bufs=2)
    for m4 in range(4):
        osb = ob.tile([128, 1024], F32, tag="osb", name="osb")
        for jj in range(2):
            poo = ps_o.tile([128, 512], F32, tag="poo", name="poo")
            for k in range(NK):
                nc.tensor.matmul(poo[:], ga_sb[:, k, 128 * m4:128 * m4 + 128],
                                 wo_sb[:, k, 512 * jj:512 * jj + 512],
                                 start=(k == 0), stop=(k == NK - 1))
            nc.vector.tensor_copy(osb[:, 512 * jj:512 * jj + 1024 - 512], poo[:])
        nc.sync.dma_start(out=t["out"].ap()[128 * m4:128 * m4 + 128, :], in_=osb[:])

    ob.release()
    ps_o.release()
    singles.release()


def build_graph():
    nc = bacc.Bacc("TRN2", target_bir_lowering=False, debug=False, num_devices=NC)
    t = {}
    t["xt"] = nc.dram_tensor("xt", [NK, 128, SEQ], BF16, kind="ExternalInput")
    t["wq2"] = nc.dram_tensor("wq2", [NK, 128, 256], BF16, kind="ExternalInput")
    t["wk2"] = nc.dram_tensor("wk2", [NK, 128, 256], BF16, kind="ExternalInput")
    t["wv"] = nc.dram_tensor("wv", [NK, 128, 128], BF16, kind="ExternalInput")
    t["wo"] = nc.dram_tensor("wo", [NK, 128, 1024], BF16, kind="ExternalInput")
    t["cost"] = nc.dram_tensor("cost", [128, SEQ], BF16, kind="ExternalInput")
    t["sint"] = nc.dram_tensor("sint", [128, SEQ], BF16, kind="ExternalInput")
    t["masks"] = nc.dram_tensor("masks", [128, 4 * 512], BF16, kind="ExternalInput")
    t["gate"] = nc.dram_tensor("gate", [64, 1], F32, kind="ExternalInput")
    t["out"] = nc.dram_tensor("out", [SEQ // NC, D], F32, kind="ExternalOutput")
    t["cc_in"] = nc.dram_tensor("cc_in", [NC * 128, 512], BF16)
    t["cc_out"] = nc.dram_tensor("cc_out", [NC * 128, 512], BF16)
    with TileContext(nc) as tc:
        _emit(nc, tc, t)
    nc.compile()
    return nc


def _bf16(a):
    return np.asarray(a, dtype=np.float32).astype(ml_dtypes.bfloat16)


def _shift_sign(w):
    """Rows p: p%64<32 -> -w[p+32]; else +w[p-32] (within each 64-row head block)."""
    out = np.empty_like(w)
    for h0 in range(0, w.shape[0], 64):
        out[h0:h0 + 32] = -w[h0 + 32:h0 + 64]
        out[h0 + 32:h0 + 64] = w[h0:h0 + 32]
    return out


def host_prep(x, Wq, Wk, Wv, Wo, gate_w):
    x = np.asarray(x, np.float32)
    Wq = np.asarray(Wq, np.float32)
    Wk = np.asarray(Wk, np.float32)
    Wv = np.asarray(Wv, np.float32)
    Wo = np.asarray(Wo, np.float32)
    gate_w = np.asarray(gate_w, np.float32)

    xt = _bf16(np.ascontiguousarray(x.reshape(SEQ, D).T).reshape(NK, 128, SEQ))
    wo = _bf16(np.ascontiguousarray(Wo.T).reshape(NK, 128, D))

    half = HD // 2
    inv_freq = 1.0 / (ROPE_BASE ** (np.arange(half, dtype=np.float32) / half))
    ang = np.arange(S, dtype=np.float32)[:, None] * inv_freq[None, :]  # [S, 32]
    cos_f = np.cos(ang)  # [S, 32]
    sin_f = np.sin(ang)
    p32 = np.arange(128) % 32
    nmod = np.tile(np.arange(S), B)
    cost = _bf16(cos_f[nmod[None, :], p32[:, None]])
    sint = _bf16(sin_f[nmod[None, :], p32[:, None]])

    p = np.arange(128)[:, None]
    qp = np.arange(512)[None, :]
    masks = np.zeros((128, 4, 512), np.float32)
    for o in range(4):
        masks[:, o, :] = (128 * o + p <= qp)
    masks = _bf16(masks.reshape(128, 4 * 512))

    gate = (HADAMARD_SCALE * gate_w).astype(np.float32).reshape(64, 1)

    in_maps = []
    for c in range(NC):
        hs = slice(128 * c, 128 * c + 128)
        wq_s = Wq[hs] * INV_SQRT_HD
        wk_s = Wk[hs]
        wq2 = np.concatenate([
            np.ascontiguousarray(wq_s.T).reshape(NK, 128, 128),
            np.ascontiguousarray(_shift_sign(wq_s).T).reshape(NK, 128, 128),
        ], axis=2)
        wk2 = np.concatenate([
            np.ascontiguousarray(wk_s.T).reshape(NK, 128, 128),
            np.ascontiguousarray(_shift_sign(wk_s).T).reshape(NK, 128, 128),
        ], axis=2)
        wv_c = np.ascontiguousarray(Wv[hs].T).reshape(NK, 128, 128)
        in_maps.append({
            "xt": xt, "wq2": _bf16(wq2), "wk2": _bf16(wk2), "wv": _bf16(wv_c),
            "wo": wo, "cost": cost, "sint": sint, "masks": masks, "gate": gate,
        })
    return in_maps


def _install_ntff_shim():
    """The agent image's antenv lacks axon_hooks; recreate it so
    run_bass_kernel_spmd(trace=True) can capture an NTFF profile."""
    import sys
    import types
    if "antenv.axon_hooks" in sys.modules:
        return True
    try:
        import antenv  # noqa: F401
        from trn_agent_boot.trn_boot import _ntff_profile_via_ctypes
        mod = types.ModuleType("antenv.axon_hooks")
        mod._hook = None
        mod.set_axon_ntff_profile_hook = lambda h: setattr(mod, "_hook", h)
        mod.get_axon_ntff_profile_hook = lambda: mod._hook
        sys.modules["antenv.axon_hooks"] = mod
        mod.set_axon_ntff_profile_hook(_ntff_profile_via_ctypes("/opt/axon/libaxon_pjrt.so"))
        import concourse.bass_utils as bu
        bu.upload_artifacts = lambda tmpdir: str(tmpdir)
        return True
    except Exception:
        return False


def kernel(x, Wq, Wk, Wv, Wo, gate_w):
    global _GRAPH
    if _GRAPH is None:
        _GRAPH = build_graph()
    in_maps = host_prep(x, Wq, Wk, Wv, Wo, gate_w)
    trace = bool(os.environ.get("KERNEL_TRACE")) and _install_ntff_shim()
    res = run_bass_kernel_spmd(_GRAPH, in_maps, core_ids=list(range(NC)), trace=trace)
    if trace and res.exec_time_ns is not None:
        print(f"HW exec time: {res.exec_time_ns} ns")
        kernel.last_exec_time_ns = res.exec_time_ns
        kernel.last_profile = res
    out = np.concatenate([res.results[c]["out"] for c in range(NC)], axis=0)
    return out.reshape(B, S, D)


# revision 10
# speedup vs baseline: 1.0937x; 1.0937x over previous
"""AmplifiedAttention Trainium2 kernel (8 NeuronCores, SPMD).

Key algebraic simplification: rotate_half(q)·rotate_half(k) == q·k and
(rotate_half(q)^2)·(rotate_half(k)^2) == q^2·k^2, so the reference's second
"rotated" attention pass is bit-for-bit (up to fp assoc.) identical to the
first: out = out1 + HADAMARD_SCALE * out1^2 * gate_w.

Sharding: 16 heads -> 2 heads/core. Each core projects Q/K/V for its heads
over the full (batch*seq) axis, runs causal attention with a fused
second-order score term ([rope(Q); sqrt(lam)*rope(Q)^2] contraction), then an
AllToAll redistributes attention outputs so each core computes a 512-row
slice of the final output projection against the full Wo.
"""

import math
import os

import numpy as np
import ml_dtypes

import concourse.bass as bass
import concourse.bacc as bacc
import concourse.mybir as mybir
from concourse.tile import TileContext
from concourse.bass_utils import run_bass_kernel_spmd

BF16 = mybir.dt.bfloat16
F32 = mybir.dt.float32

B, S, D = 2, 2048, 1024
H = 16
HD = D // H            # 64
NC = 8                 # cores
NHPC = H // NC         # 2 heads per core
SEQ = B * S            # 4096
NK = D // 128          # 8 contraction chunks
NNT = SEQ // 512       # 8 seq 512-tiles
NST = SEQ // 128       # 32 seq 128-tiles
NQT = S // 512         # 4 query 512-tiles per (b,h)
NTC = S // 128         # 16 key 128-chunks per (b,h)

LAMBDA = 0.1
HADAMARD_SCALE = 0.05
ROPE_BASE = 10000.0
INV_SQRT_HD = 1.0 / math.sqrt(HD)
# Q weights are pre-scaled by INV_SQRT_HD on the host.  Q-tilde bottom rows
# need sqrt(lam)*INV_SQRT_HD*ropeQ^2 computed from the pre-scaled ropeQ:
#   (lam^0.25 / sqrt(INV_SQRT_HD) * x)^2 = sqrt(lam)/INV_SQRT_HD * x^2
Q_SQ_SCALE = LAMBDA ** 0.25 / math.sqrt(INV_SQRT_HD)
K_SQ_SCALE = LAMBDA ** 0.25

_GRAPH = None


def _emit(nc, tc, t):
    """Emit the per-core program. t: dict name -> DRAM tensor handle."""
    AF = mybir.ActivationFunctionType
    OP = mybir.AluOpType
    singles = tc.alloc_tile_pool(name="singles", bufs=1)

    # ---- constant / input loads ----
    cos_sb = singles.tile([128, SEQ], BF16, tag="cos", name="cos")
    sin_sb = singles.tile([128, SEQ], BF16, tag="sin", name="sin")
    masks_sb = singles.tile([128, 4, 512], BF16, tag="masks", name="masks")
    gate_sb = singles.tile([64, 1], F32, tag="gate", name="gate")
    wq_sb = singles.tile([128, NK, 256], BF16, tag="wq", name="wq")
    wk_sb = singles.tile([128, NK, 256], BF16, tag="wk", name="wk")
    wv_sb = singles.tile([128, NK, 128], BF16, tag="wv", name="wv")
    wo_sb = singles.tile([128, NK, 128], BF16, tag="wo", name="wo")
    xt_sb = singles.tile([128, NK, SEQ], BF16, tag="xt", name="xt")

    nc.sync.dma_start(out=cos_sb[:], in_=t["cost"].ap())
    nc.sync.dma_start(out=sin_sb[:], in_=t["sint"].ap())
    nc.sync.dma_start(out=masks_sb[:], in_=t["masks"].ap().rearrange("p (o q) -> p o q", o=4))
    nc.sync.dma_start(out=gate_sb[:], in_=t["gate"].ap())
    nc.sync.dma_start(out=wq_sb[:], in_=t["wq2"].ap().rearrange("k p m -> p k m"))
    nc.sync.dma_start(out=wk_sb[:], in_=t["wk2"].ap().rearrange("k p m -> p k m"))
    nc.sync.dma_start(out=wv_sb[:], in_=t["wv"].ap().rearrange("k p m -> p k m"))
    nc.sync.dma_start(out=wo_sb[:], in_=t["wo"].ap().rearrange("k p j -> p k j"))
    for k in range(NK):
        nc.sync.dma_start(out=xt_sb[:, k, :], in_=t["xt"].ap()[k])

    # ---- persistent stage-1 outputs ----
    # qt/kt per (b, h): [128, S]: rows 0:64 = rope (Q pre-scaled by 1/sqrt(hd)),
    # rows 64:128 = scaled square.
    qt = [[singles.tile([128, S], BF16, tag=f"qt{b}{h}", name=f"qt{b}{h}") for h in range(NHPC)] for b in range(B)]
    kt = [[singles.tile([128, S], BF16, tag=f"kt{b}{h}", name=f"kt{b}{h}") for h in range(NHPC)] for b in range(B)]
    # v_sb: [128, NST, 256]: per 128-seq-chunk: [ones(0:64), h0(64:128), ones(128:192), h1(192:256)]
    v_sb = singles.tile([128, NST, 256], BF16, tag="v", name="v")
    att_sb = singles.tile([128, SEQ], BF16, tag="att", name="att")

    ones_ap = bass.AP(
        tensor=v_sb.tensor, offset=v_sb.offset,
        ap=[v_sb.ap[0], [256, NST], [128, 2], [1, 64]],
    )
    nc.vector.memset(ones_ap, 1.0)

    # ---- stage 1: projections + rope + squares + V ----
    ps_proj = tc.alloc_tile_pool(name="ps_proj", bufs=6, space="PSUM")
    ps_v = tc.alloc_tile_pool(name="ps_v", bufs=2, space="PSUM")
    rope_tmp = tc.alloc_tile_pool(name="rope_tmp", bufs=4)

    for n in range(NNT):
        ns = slice(512 * n, 512 * n + 512)
        b, j2 = n // NQT, n % NQT
        cs = slice(512 * j2, 512 * j2 + 512)
        for (w2, dst, ss) in ((wq_sb, qt, None), (wk_sb, kt, None)):
            ps_a = ps_proj.tile([128, 512], F32, tag="pp", name="pp")
            ps_s = ps_proj.tile([128, 512], F32, tag="pp", name="pp")
            for k in range(NK):
                nc.tensor.matmul(ps_a[:], w2[:, k, 0:128], xt_sb[:, k, ns],
                                 start=(k == 0), stop=(k == NK - 1))
            for k in range(NK):
                nc.tensor.matmul(ps_s[:], w2[:, k, 128:256], xt_sb[:, k, ns],
                                 start=(k == 0), stop=(k == NK - 1))
            pc = rope_tmp.tile([128, 512], BF16, tag="pc", name="pc")
            psn = rope_tmp.tile([128, 512], BF16, tag="psn", name="psn")
            nc.vector.tensor_tensor(out=pc[:], in0=ps_a[:], in1=cos_sb[:, ns], op=OP.mult)
            nc.vector.tensor_tensor(out=psn[:], in0=ps_s[:], in1=sin_sb[:, ns], op=OP.mult)
            nc.vector.tensor_tensor(out=dst[b][0][0:64, cs], in0=pc[0:64, :], in1=psn[0:64, :], op=OP.add)
            nc.vector.tensor_tensor(out=dst[b][1][0:64, cs], in0=pc[64:128, :], in1=psn[64:128, :], op=OP.add)
        # V for the 4 seq-128-chunks of this n-tile
        for st4 in range(4):
            st = 4 * n + st4
            pv = ps_v.tile([128, 128], F32, tag="pv", name="pv")
            for k in range(NK):
                nc.tensor.matmul(pv[:], xt_sb[:, k, 128 * st:128 * st + 128], wv_sb[:, k, :],
                                 start=(k == 0), stop=(k == NK - 1))
            vdst = bass.AP(
                tensor=v_sb.tensor, offset=v_sb.offset + 256 * st + 64,
                ap=[v_sb.ap[0], [128, 2], [1, 64]],
            )
            nc.scalar.copy(vdst, pv[:].rearrange("p (a b) -> p a b", a=2))

    for b in range(B):
        for h in range(NHPC):
            nc.vector.scalar_tensor_tensor(
                out=qt[b][h][64:128, :], in0=qt[b][h][0:64, :], scalar=Q_SQ_SCALE ** 2,
                in1=qt[b][h][0:64, :], op0=OP.mult, op1=OP.mult)
            nc.vector.scalar_tensor_tensor(
                out=kt[b][h][64:128, :], in0=kt[b][h][0:64, :], scalar=K_SQ_SCALE ** 2,
                in1=kt[b][h][0:64, :], op0=OP.mult, op1=OP.mult)

    rope_tmp.release()
    ps_v.release()
    ps_proj.release()

    # ---- stage 2: attention ----
    ps_s = tc.alloc_tile_pool(name="ps_s", bufs=3, space="PSUM")
    ps_av = tc.alloc_tile_pool(name="ps_av", bufs=2, space="PSUM")
    a_pool = tc.alloc_tile_pool(name="a_pool", bufs=8)
    nrm = tc.alloc_tile_pool(name="nrm", bufs=3)

    for b in range(B):
        for h in range(NHPC):
            QT, KT = qt[b][h], kt[b][h]
            for j in range(NQT):
                qs = slice(512 * j, 512 * j + 512)
                I = 4 * j + 4  # t-chunks (causal keep)
                po = ps_av.tile([128, 512], F32, tag="po", name="po")
                # pass 1: scores + exp + mask for all t-chunk pairs, so the
                # PE streams score matmuls without stalling on ACT's exp
                a_list = []
                for ip in range(I // 2):
                    i0, i1 = 2 * ip, 2 * ip + 1
                    pss = ps_s.tile([128, 1024], F32, tag="pss", name="pss")
                    nc.tensor.matmul(pss[:, 0:512], KT[:, 128 * i0:128 * i0 + 128], QT[:, qs],
                                     start=True, stop=True)
                    nc.tensor.matmul(pss[:, 512:1024], KT[:, 128 * i1:128 * i1 + 128], QT[:, qs],
                                     start=True, stop=True)
                    a = a_pool.tile([128, 1024], BF16, tag="a", name="a")
                    nc.scalar.activation(out=a[:], in_=pss[:], func=AF.Exp)
                    if i0 >= 4 * j:  # diagonal pair -> apply causal masks
                        nc.vector.tensor_tensor(out=a[:, 0:512], in0=a[:, 0:512],
                                                in1=masks_sb[:, i0 - 4 * j, :], op=OP.mult)
                        nc.vector.tensor_tensor(out=a[:, 512:1024], in0=a[:, 512:1024],
                                                in1=masks_sb[:, i1 - 4 * j, :], op=OP.mult)
                    a_list.append(a)
                # pass 2: A@V accumulation
                for ip, a in enumerate(a_list):
                    i0, i1 = 2 * ip, 2 * ip + 1
                    nc.tensor.matmul(po[:], v_sb[:, 16 * b + i0, 128 * h:128 * h + 128], a[:, 0:512],
                                     start=(ip == 0), stop=False)
                    nc.tensor.matmul(po[:], v_sb[:, 16 * b + i1, 128 * h:128 * h + 128], a[:, 512:1024],
                                     start=False, stop=(ip == I // 2 - 1))
                # rows 0:64 of po = softmax denominators (replicated), 64:128 = A@V
                rd = nrm.tile([64, 512], F32, tag="rd", name="rd")
                m = nrm.tile([64, 512], BF16, tag="m", name="m")
                sq = nrm.tile([64, 512], BF16, tag="sq", name="sq")
                nc.vector.reciprocal_approx_fast(out=rd[:], in_=po[0:64, :])
                nc.vector.tensor_tensor(out=m[:], in0=po[64:128, :], in1=rd[:], op=OP.mult)
                nc.vector.tensor_tensor(out=sq[:], in0=m[:], in1=m[:], op=OP.mult)
                nc.vector.scalar_tensor_tensor(
                    out=att_sb[64 * h:64 * h + 64, 2048 * b + 512 * j:2048 * b + 512 * j + 512],
                    in0=sq[:], scalar=gate_sb[:, 0:1], in1=m[:],
                    op0=OP.mult, op1=OP.add)
        # batch b fully done on this core -> AllGather its attention columns
        # while the next batch computes.
        nc.sync.dma_start(out=t[f"ag_in{b}"].ap(), in_=att_sb[:, 2048 * b:2048 * b + 2048])
        nc.gpsimd.collective_compute(
            "AllGather", OP.bypass,
            replica_groups=[list(range(NC))],
            ins=[t[f"ag_in{b}"].ap()], outs=[t[f"ag_out{b}"].ap()],
        )

    nrm.release()
    a_pool.release()
    ps_av.release()
    ps_s.release()

    # ---- stage 3: column-sharded output projection ----
    # Each core computes out[j=128c:128c+128, all seq] = Wo_slice @ attn_all
    # (transposed layout), so the gathered read is rank-independent.
    ps_o = tc.alloc_tile_pool(name="ps_o", bufs=8, space="PSUM")
    ga_pool = tc.alloc_tile_pool(name="ga_pool", bufs=3)
    ob = tc.alloc_tile_pool(name="ob", bufs=3)
    for b in range(B):
        pos = [ps_o.tile([128, 512], F32, tag="poo", name="poo") for _ in range(4)]
        for k in range(NK):
            ga = ga_pool.tile([128, 2048], BF16, tag="ga", name="ga")
            nc.sync.dma_start(out=ga[:], in_=t[f"ag_out{b}"].ap()[128 * k:128 * k + 128, :])
            for st in range(4):
                nc.tensor.matmul(pos[st][:], wo_sb[:, k, :], ga[:, 512 * st:512 * st + 512],
                                 start=(k == 0), stop=(k == NK - 1))
        for st in range(4):
            osb = ob.tile([128, 512], F32, tag="osb", name="osb")
            nc.vector.tensor_copy(osb[:], pos[st][:])
            nc.sync.dma_start(
                out=t["out"].ap()[:, 2048 * b + 512 * st:2048 * b + 512 * st + 512],
                in_=osb[:])

    ob.release()
    ga_pool.release()
    ps_o.release()
    singles.release()


def build_graph():
    nc = bacc.Bacc("TRN2", target_bir_lowering=False, debug=False, num_devices=NC)
    t = {}
    t["xt"] = nc.dram_tensor("xt", [NK, 128, SEQ], BF16, kind="ExternalInput")
    t["wq2"] = nc.dram_tensor("wq2", [NK, 128, 256], BF16, kind="ExternalInput")
    t["wk2"] = nc.dram_tensor("wk2", [NK, 128, 256], BF16, kind="ExternalInput")
    t["wv"] = nc.dram_tensor("wv", [NK, 128, 128], BF16, kind="ExternalInput")
    t["wo"] = nc.dram_tensor("wo", [NK, 128, 128], BF16, kind="ExternalInput")
    t["cost"] = nc.dram_tensor("cost", [128, SEQ], BF16, kind="ExternalInput")
    t["sint"] = nc.dram_tensor("sint", [128, SEQ], BF16, kind="ExternalInput")
    t["masks"] = nc.dram_tensor("masks", [128, 4 * 512], BF16, kind="ExternalInput")
    t["gate"] = nc.dram_tensor("gate", [64, 1], F32, kind="ExternalInput")
    t["out"] = nc.dram_tensor("out", [128, SEQ], F32, kind="ExternalOutput")
    for b in range(B):
        t[f"ag_in{b}"] = nc.dram_tensor(f"ag_in{b}", [128, S], BF16)
        t[f"ag_out{b}"] = nc.dram_tensor(f"ag_out{b}", [NC * 128, S], BF16, addr_space="Shared")
    with TileContext(nc) as tc:
        _emit(nc, tc, t)
    nc.compile()
    return nc


def _bf16(a):
    return np.asarray(a, dtype=np.float32).astype(ml_dtypes.bfloat16)


def _shift_sign(w):
    """Rows p: p%64<32 -> -w[p+32]; else +w[p-32] (within each 64-row head block)."""
    out = np.empty_like(w)
    for h0 in range(0, w.shape[0], 64):
        out[h0:h0 + 32] = -w[h0 + 32:h0 + 64]
        out[h0 + 32:h0 + 64] = w[h0:h0 + 32]
    return out


def host_prep(x, Wq, Wk, Wv, Wo, gate_w):
    x = np.asarray(x, np.float32)
    Wq = np.asarray(Wq, np.float32)
    Wk = np.asarray(Wk, np.float32)
    Wv = np.asarray(Wv, np.float32)
    Wo = np.asarray(Wo, np.float32)
    gate_w = np.asarray(gate_w, np.float32)

    xt = _bf16(np.ascontiguousarray(x.reshape(SEQ, D).T).reshape(NK, 128, SEQ))

    half = HD // 2
    inv_freq = 1.0 / (ROPE_BASE ** (np.arange(half, dtype=np.float32) / half))
    ang = np.arange(S, dtype=np.float32)[:, None] * inv_freq[None, :]  # [S, 32]
    cos_f = np.cos(ang)  # [S, 32]
    sin_f = np.sin(ang)
    p32 = np.arange(128) % 32
    nmod = np.tile(np.arange(S), B)
    cost = _bf16(cos_f[nmod[None, :], p32[:, None]])
    sint = _bf16(sin_f[nmod[None, :], p32[:, None]])

    p = np.arange(128)[:, None]
    qp = np.arange(512)[None, :]
    masks = np.zeros((128, 4, 512), np.float32)
    for o in range(4):
        masks[:, o, :] = (128 * o + p <= qp)
    masks = _bf16(masks.reshape(128, 4 * 512))

    gate = (HADAMARD_SCALE * gate_w).astype(np.float32).reshape(64, 1)

    in_maps = []
    for c in range(NC):
        hs = slice(128 * c, 128 * c + 128)
        wq_s = Wq[hs] * INV_SQRT_HD
        wk_s = Wk[hs]
        wq2 = np.concatenate([
            np.ascontiguousarray(wq_s.T).reshape(NK, 128, 128),
            np.ascontiguousarray(_shift_sign(wq_s).T).reshape(NK, 128, 128),
        ], axis=2)
        wk2 = np.concatenate([
            np.ascontiguousarray(wk_s.T).reshape(NK, 128, 128),
            np.ascontiguousarray(_shift_sign(wk_s).T).reshape(NK, 128, 128),
        ], axis=2)
        wv_c = np.ascontiguousarray(Wv[hs].T).reshape(NK, 128, 128)
        wo_c = np.ascontiguousarray(Wo[hs].T).reshape(NK, 128, 128)
        in_maps.append({
            "xt": xt, "wq2": _bf16(wq2), "wk2": _bf16(wk2), "wv": _bf16(wv_c),
            "wo": _bf16(wo_c), "cost": cost, "sint": sint, "masks": masks, "gate": gate,
        })
    return in_maps


def _install_ntff_shim():
    """The agent image's antenv lacks axon_hooks; recreate it so
    run_bass_kernel_spmd(trace=True) can capture an NTFF profile."""
    import sys
    import types
    if "antenv.axon_hooks" in sys.modules:
        return True
    try:
        import antenv  # noqa: F401
        from trn_agent_boot.trn_boot import _ntff_profile_via_ctypes
        mod = types.ModuleType("antenv.axon_hooks")
        mod._hook = None
        mod.set_axon_ntff_profile_hook = lambda h: setattr(mod, "_hook", h)
        mod.get_axon_ntff_profile_hook = lambda: mod._hook
        sys.modules["antenv.axon_hooks"] = mod
        mod.set_axon_ntff_profile_hook(_ntff_profile_via_ctypes("/opt/axon/libaxon_pjrt.so"))
        import concourse.bass_utils as bu
        bu.upload_artifacts = lambda tmpdir: str(tmpdir)
        return True
    except Exception:
        return False


def kernel(x, Wq, Wk, Wv, Wo, gate_w):
    global _GRAPH
    if _GRAPH is None:
        _GRAPH = build_graph()
    in_maps = host_prep(x, Wq, Wk, Wv, Wo, gate_w)
    trace = bool(os.environ.get("KERNEL_TRACE")) and _install_ntff_shim()
    res = run_bass_kernel_spmd(_GRAPH, in_maps, core_ids=list(range(NC)), trace=trace)
    if trace and res.exec_time_ns is not None:
        print(f"HW exec time: {res.exec_time_ns} ns")
        kernel.last_exec_time_ns = res.exec_time_ns
        kernel.last_profile = res
    out = np.empty((SEQ, D), np.float32)
    for c in range(NC):
        out[:, 128 * c:128 * c + 128] = res.results[c]["out"].T
    return out.reshape(B, S, D)
